# revision 38
# baseline (speedup 1.0000x reference)
"""Causal multi-head attention on 8 TRN2 NeuronCores.

Problem: B=4, T=2048, d_model=1024, 16 heads x 64. out = softmax(causal(QK^T)/8) V Wo.

Sharding (tensor-parallel heads x data-parallel batch):
  core c -> batch b = c//2, head group g = c%2 (8 heads each).
  Each core computes a partial output  z_g[b] @ Wo[g] : [2048, 1024];
  host sums the two head-group partials per batch.

Per-core kernel. PE and ACT are co-critical (~210us each): the ACT exp
stream (one exp per score element + the Ln->Exp 1/D chain) is a hard floor,
so PE work is trimmed to just above it -- going lower only re-throttles the
PE clock (HAM) during the idle gaps and makes everything slower:
  - Projections stay bf16 (they pace the pipeline and keep the PE warm).
  - AV off-diagonal key-block pairs run as single fp8e4 DoubleRow matmuls
    (HW-measured 2.0x: both key blocks contract in one 512-column stream).
    The ACT exp writes those attention weights straight to fp8; V tiles are
    kept as fp8 32*V pairs (slot stride 576B satisfies DR's 16B rule).
    Diagonal pairs stay 2-matmul (fp8 non-DR ch1-3, bf16 ch0) so causal
    dead zones are never streamed; chunk 0 stays bf16 because its small-Neff
    rows are sensitive to weight/V noise. Mixed DR/non-DR psum accumulation
    groups are HW-validated.
  - Output projection chunks 1-3: zt as fp8 64z x Wo as fp8 4*Wo, DoubleRow
    over head-dim pairs; psum = 256*out, rescaled in the DVE copy. Chunk 0
    stays f32r (zt = 64z there too; copy rescales by 1/64).
  Scale plumbing: V copies apply x32 (e4m3's normal range starts at 2^-6);
  the 1/D broadcast uses ones=2.0 so zt = u * (2/D) = 64z uniformly.
Baseline schedule retained: per-head attention pipeline, AV two windows
behind scores, proj/outproj fills interleaved, deferred 1/D chain.
"""
import numpy as np

import concourse.bass as bass
import concourse.tile as tile
import concourse.mybir as mybir
from concourse.vector_clock import ScopedClock
from concourse.bass_utils import run_bass_kernel_spmd

D_MODEL = 1024
D_HEAD = 64
B = 4
T = 2048
H = 8              # heads per core
HG = H * D_HEAD    # 512 head-dim columns per core
TCH = 512          # q/t chunk
NCH = T // TCH     # 4
NDM = D_MODEL // 128  # 8 d_model chunks

F32R = mybir.dt.float32r
F32 = mybir.dt.float32
BF16 = mybir.dt.bfloat16
FP8 = mybir.dt.float8e4
AF = mybir.ActivationFunctionType
DR = mybir.MatmulPerfMode.DoubleRow
ESC = 0.125 / 1024.0   # exp scale: scores are (32q)(32k) = 1024*S


class _TC(tile.TileContext):
    """TileContext whose tail drain carries no sem waits (this walrus build
    rejects >1 sync wait per instruction and any wait on a Drain)."""

    def _drain_and_barrier(self, tick_clock, wait_clock):
        drain_inst = self.nc.sync.drain()
        wait_clock.add_sem_waits(
            drain_inst.ins, ScopedClock({None: tick_clock.global_clock})
        )
        si = drain_inst.ins.sync_info
        waits = list(si.on_wait) if si is not None else []
        if waits:
            drain_inst.ins.sync_info = mybir.SyncInfo(
                on_wait=[], on_update=list(si.on_update)
            )
            for w in waits:
                nop = self.nc.sync.nop(nofuse=True)
                nop.ins.sync_info = mybir.SyncInfo(on_wait=[w], on_update=[])
        self.nc.all_engine_barrier()
        popped = self.nc._tile_sem_poison_stack.pop()
        assert popped is self._sem_poison
        self.nc.clear_and_free_semaphores(list(self.sems.allocated().values()))
        self.nc.all_engine_barrier()


def _split_multi_waits(nc):
    """Move all-but-one sem wait of every instruction onto same-engine NOPs."""
    cnt = 0
    for f in nc.m.functions:
        for b in f.blocks:
            new = []
            for inst in b.instructions:
                si = inst.sync_info
                if si is not None and si.on_wait is not None:
                    waits = list(si.on_wait)
                    max_keep = 0 if inst.opcode == "Drain" else 1
                    if len(waits) > max_keep:
                        keep = waits[len(waits) - max_keep:] if max_keep else []
                        spill = waits[: len(waits) - max_keep]
                        for w in spill:
                            nop = mybir.InstNoOp(
                                name=f"I-wsplit-{cnt}", engine=inst.engine,
                                ins=[], outs=[],
                            )
                            nop.sync_info = mybir.SyncInfo(
                                on_wait=[w], on_update=[]
                            )
                            new.append(nop)
                            cnt += 1
                        inst.sync_info = mybir.SyncInfo(
                            on_wait=keep, on_update=list(si.on_update)
                        )
                new.append(inst)
            b.instructions = new
    return cnt


def _build():
    nc = bass.Bass("TRN2", target_bir_lowering=False)
    xT = nc.dram_tensor("xT", (D_MODEL, T), BF16, kind="ExternalInput")
    xT8 = nc.dram_tensor("xT8", (D_MODEL, T), FP8, kind="ExternalInput")
    wq = nc.dram_tensor("wq", (D_MODEL, HG), BF16, kind="ExternalInput")
    wk = nc.dram_tensor("wk", (D_MODEL, HG), BF16, kind="ExternalInput")
    wv = nc.dram_tensor("wv", (D_MODEL, HG), BF16, kind="ExternalInput")
    wq8 = nc.dram_tensor("wq8", (D_MODEL, HG), FP8, kind="ExternalInput")
    wk8 = nc.dram_tensor("wk8", (D_MODEL, HG), FP8, kind="ExternalInput")
    wv8 = nc.dram_tensor("wv8", (D_MODEL, HG), FP8, kind="ExternalInput")
    wo = nc.dram_tensor("wo", (HG, D_MODEL), F32R, kind="ExternalInput")
    wo8 = nc.dram_tensor("wo8", (HG, D_MODEL), FP8, kind="ExternalInput")
    tri = nc.dram_tensor("tri", (128, 128), BF16, kind="ExternalInput")
    ones1 = nc.dram_tensor("ones1", (1, 64), F32R, kind="ExternalInput")
    vones_b = nc.dram_tensor("vones_b", (128, 4, H, 1), BF16,
                             kind="ExternalInput")
    vones_8 = nc.dram_tensor("vones_8", (128, T // 128, H, 1), FP8,
                             kind="ExternalInput")
    out = nc.dram_tensor("out", (T, D_MODEL), F32, kind="ExternalOutput")

    from contextlib import ExitStack
    with _TC(nc) as tc, ExitStack() as ctx:
        consts = ctx.enter_context(tc.tile_pool(name="consts", bufs=1))
        xs_pool = ctx.enter_context(tc.tile_pool(name="xs", bufs=3))
        kt_pool = ctx.enter_context(tc.tile_pool(name="kt", bufs=1))
        v_pool = ctx.enter_context(tc.tile_pool(name="v", bufs=1))
        qt_pool = ctx.enter_context(tc.tile_pool(name="qt", bufs=3))
        ztr_pool = ctx.enter_context(tc.tile_pool(name="ztr", bufs=1))
        zt8_pool = ctx.enter_context(tc.tile_pool(name="zt8", bufs=3))
        etb_pool = ctx.enter_context(tc.tile_pool(name="etb", bufs=4))
        et8_pool = ctx.enter_context(tc.tile_pool(name="et8", bufs=6))
        sm_pool = ctx.enter_context(tc.tile_pool(name="sm", bufs=2))
        rb_pool = ctx.enter_context(tc.tile_pool(name="rb", bufs=2))
        ou_pool = ctx.enter_context(tc.tile_pool(name="ou", bufs=2))
        # PSUM: 8 banks. s2 tiles [128,2,512]f32 = 2 banks x 2 bufs;
        # u tiles 1 bank x 2; w tiles (proj/outproj/bcast) 1 x 2.
        ps_s = ctx.enter_context(tc.tile_pool(name="ps_s", bufs=2, space="PSUM"))
        ps_u = ctx.enter_context(tc.tile_pool(name="ps_u", bufs=2, space="PSUM"))
        ps_w = ctx.enter_context(tc.tile_pool(name="ps_w", bufs=2, space="PSUM"))

        xT_r = xT.ap().rearrange("(c p) t -> p c t", p=128)
        xT8_r = xT8.ap().rearrange("(c p) t -> p c t", p=128)

        xs_tiles = [None] * NCH

        def dma_xs(ch):
            if ch == 1:
                xs_tiles[ch] = xs_pool.tile([128, NDM, TCH], FP8, name="xs8",
                                            tag="xs8")
                for c in range(0, NDM, 4):
                    nc.sync.dma_start(out=xs_tiles[ch][:, c:c + 4, :],
                                      in_=xT8_r[:, c:c + 4,
                                               ch * TCH:(ch + 1) * TCH])
            else:
                xs_tiles[ch] = xs_pool.tile([128, NDM, TCH], BF16, name="xs",
                                            tag="xs")
                for c in range(0, NDM, 2):
                    nc.sync.dma_start(out=xs_tiles[ch][:, c:c + 2, :],
                                      in_=xT_r[:, c:c + 2,
                                               ch * TCH:(ch + 1) * TCH])

        # resident weights / constants, DMA'd in need-order on the serial
        # sync queue: proj(0)'s bf16 wq/x/wk first, then chunk-0's fill
        # inputs (fp8 x1 + fp8 q/k weights), then the V weights, constants,
        # x2; wo/wo8 are deferred to chunk 1 (first read mid-chunk 2).
        wq_sb = consts.tile([128, NDM, HG], BF16)
        xs0 = xs_pool.tile([128, NDM, TCH], BF16, name="xs", tag="xs")
        wk_sb = consts.tile([128, NDM, HG], BF16)
        wv_sb = consts.tile([128, NDM, HG], BF16)
        wq_r = wq.ap().rearrange("(c p) n -> p c n", p=128)
        wk_r = wk.ap().rearrange("(c p) n -> p c n", p=128)
        # 2-chunk DMAs: the sync queue costs ~600ns per descriptor no matter
        # the size, so fewer+bigger descriptors feed proj(0) faster
        for c in range(0, NDM, 2):
            nc.sync.dma_start(out=wq_sb[:, c:c + 2, :], in_=wq_r[:, c:c + 2, :])
            nc.sync.dma_start(out=xs0[:, c:c + 2, :],
                              in_=xT_r[:, c:c + 2, 0:TCH])
            nc.sync.dma_start(out=wk_sb[:, c:c + 2, :], in_=wk_r[:, c:c + 2, :])
        xs_tiles[0] = xs0
        wq8_sb = consts.tile([128, NDM, HG], FP8)
        wk8_sb = consts.tile([128, NDM, HG], FP8)
        wv8_sb = consts.tile([128, NDM, HG], FP8)
        dma_xs(1)
        nc.sync.dma_start(out=wq8_sb,
                          in_=wq8.ap().rearrange("(c p) n -> p c n", p=128))
        nc.sync.dma_start(out=wk8_sb,
                          in_=wk8.ap().rearrange("(c p) n -> p c n", p=128))
        nc.sync.dma_start(out=wv_sb,
                          in_=wv.ap().rearrange("(c p) n -> p c n", p=128))
        nc.sync.dma_start(out=wv8_sb,
                          in_=wv8.ap().rearrange("(c p) n -> p c n", p=128))
        tri_sb = consts.tile([128, 128], BF16)
        nc.sync.dma_start(out=tri_sb, in_=tri.ap())
        ones_sb = consts.tile([1, 64], F32R)
        nc.sync.dma_start(out=ones_sb, in_=ones1.ap())
        wo_sb = consts.tile([128, HG // 128, D_MODEL], F32R)
        wo8_sb = consts.tile([128, HG // 128, D_MODEL], FP8)

        # per-chunk K^T tiles [pair-packed 128, head-pair, t-in-chunk];
        # fp8 V pair-tiles [128, pair, slot, H, 72] (slot stride 576 % 16 == 0
        # for DoubleRow; col 64 = ones so row 64 of U accumulates the softmax
        # denominator); bf16 V tile for chunk 0 only.
        kt_tiles = [kt_pool.tile([128, 4, TCH], BF16, name=f"kt{i}", tag=f"kt{i}")
                    for i in range(NCH)]
        v8_tiles = [v_pool.tile([128, 2, 2, H, 72], FP8, name=f"v8_{i}",
                                tag=f"v8_{i}") for i in range(NCH)]
        vb0_tile = consts.tile([128, 4, H, D_HEAD + 1], BF16)
        vo8_r = vones_8.ap().rearrange("p (a c d) h o -> p a c d h o", c=2, d=2)
        for i in range(NCH):
            nc.sync.dma_start(out=v8_tiles[i][:, :, :, :, D_HEAD:D_HEAD + 1],
                              in_=vo8_r[:, i])
        nc.sync.dma_start(out=vb0_tile[:, :, :, D_HEAD:], in_=vones_b.ap())
        dma_xs(2)
        nc.sync.dma_start(out=wo_sb,
                          in_=wo.ap().rearrange("(c p) n -> p c n", p=128))
        nc.sync.dma_start(out=wo8_sb,
                          in_=wo8.ap().rearrange("(c p) n -> p c n", p=128))

        # ---- fill atoms: proj / outproj work split into single-PE-op units
        # Weights are pre-scaled 32x on the host (both dtypes) so all psum
        # projections are 32*(q|k|v) regardless of chunk dtype.
        def proj_parts(ch, xs, qt_sb):
            """Q^T,K^T per dqc and V per tt for chunk ch; returns per-unit
            atom lists. Each atom = 1 PE mm or 1 DVE copy. Chunks 1-2 run
            fp8 DoubleRow (they fill the PE-bound chunks 0-1); chunks 0/3
            stay bf16 (0 for small-Neff accuracy, 3 for fill mass in the
            ACT-bound late chunks)."""
            state = {}
            uq, uk, uv = [], [], []
            dr = ch == 1
            if dr:
                wq_s, wk_s, wv_s = wq8_sb, wk8_sb, wv8_sb
                steps = [(2 * i, 2) for i in range(NDM // 2)]
            else:
                wq_s, wk_s, wv_s = wq_sb, wk_sb, wv_sb
                steps = [(c, 1) for c in range(NDM)]
            ns = len(steps)
            pm = DR if dr else None

            def w_lhs(w, i, dqc):
                c, nn = steps[i]
                if nn == 1:
                    return w[:, c, dqc * 128:(dqc + 1) * 128]
                return w[:, c:c + 2, dqc * 128:(dqc + 1) * 128]

            def x_rhs(i):
                c, nn = steps[i]
                return xs[:, c, :] if nn == 1 else xs[:, c:c + 2, :]

            for dqc in range(4):
                unit = []
                def a_q(i, dqc=dqc):
                    if i == 0:
                        state[('q', dqc)] = ps_w.tile([128, TCH], F32, tag="w",
                                                      name="pq")
                    nc.tensor.matmul(
                        state[('q', dqc)], lhsT=w_lhs(wq_s, i, dqc),
                        rhs=x_rhs(i), start=(i == 0), stop=(i == ns - 1),
                        perf_mode=pm)
                for i in range(ns):
                    unit.append(lambda i=i, a=a_q: a(i))
                unit.append(lambda dqc=dqc: nc.vector.tensor_copy(
                    out=qt_sb[:, dqc, :], in_=state[('q', dqc)]))
                uq.append(unit)
                unit = []
                def a_k(i, dqc=dqc):
                    if i == 0:
                        state[('k', dqc)] = ps_w.tile([128, TCH], F32, tag="w",
                                                      name="pk")
                    nc.tensor.matmul(
                        state[('k', dqc)], lhsT=w_lhs(wk_s, i, dqc),
                        rhs=x_rhs(i), start=(i == 0), stop=(i == ns - 1),
                        perf_mode=pm)
                for i in range(ns):
                    unit.append(lambda i=i, a=a_k: a(i))
                unit.append(lambda dqc=dqc: nc.vector.tensor_copy(
                    out=kt_tiles[ch][:, dqc, :], in_=state[('k', dqc)]))
                uk.append(unit)
            for tt in range(4):
                unit = []
                def a_v(i, tt=tt):
                    c, nn = steps[i]
                    if i == 0:
                        state[('v', tt)] = ps_w.tile([128, HG], F32, tag="w",
                                                     name="pv")
                    if nn == 1:
                        nc.tensor.matmul(
                            state[('v', tt)],
                            lhsT=xs[:, c, tt * 128:(tt + 1) * 128],
                            rhs=wv_s[:, c, :], start=(i == 0),
                            stop=(i == ns - 1))
                    else:
                        nc.tensor.matmul(
                            state[('v', tt)],
                            lhsT=xs[:, c:c + 2, tt * 128:(tt + 1) * 128],
                            rhs=wv_s[:, c:c + 2, :], start=(i == 0),
                            stop=(i == ns - 1), perf_mode=DR)
                for i in range(ns):
                    unit.append(lambda i=i, a=a_v: a(i))
                # psum already holds 32*V; fp8 copy always, bf16 copy
                # additionally for chunk 0 (its diagonal AV)
                def c_v8(tt=tt, ch=ch):
                    with nc.allow_low_precision("fp8 32V tiles"):
                        nc.vector.tensor_copy(
                            out=v8_tiles[ch][:, tt // 2, tt % 2, :, 0:D_HEAD],
                            in_=state[('v', tt)].rearrange("p (h d) -> p h d",
                                                           h=H))
                unit.append(c_v8)
                if ch == 0:
                    unit.append(lambda tt=tt: nc.vector.tensor_copy(
                        out=vb0_tile[:, tt, :, 0:D_HEAD],
                        in_=state[('v', tt)].rearrange("p (h d) -> p h d",
                                                       h=H)))
                uv.append(unit)
            return uq, uk, uv

        def outproj_atoms(ch, zt_sb):
            atoms = []
            q0 = ch * TCH
            state = {}
            if ch == 0:
                ksteps = [(kc, 1) for kc in range(4)]
                w_s, pm, oscale = wo_sb, None, 1.0 / 64
            elif ch == NCH - 1:
                # epilogue outproj: fastest (DoubleRow)
                ksteps = [(0, 2), (2, 2)]
                w_s, pm, oscale = wo8_sb, DR, 1.0 / 256
            else:
                # fp8 non-DR: same psum scale as DR but twice the matmuls --
                # these fill the ACT-bound chunks 2-3 and keep the PE warm
                ksteps = [(kc, 1) for kc in range(4)]
                w_s, pm, oscale = wo8_sb, None, 1.0 / 256
            nk = len(ksteps)
            for tt in range(4):
                def a_alloc(tt=tt):
                    state[('o', tt)] = ou_pool.tile([128, D_MODEL], F32,
                                                    name="o_sb", tag="o")
                atoms.append(a_alloc)
                for dc in range(2):
                    def a_mm(i, tt=tt, dc=dc):
                        kc, nn = ksteps[i]
                        if i == 0:
                            state[('p', tt, dc)] = ps_w.tile(
                                [128, 512], F32, tag="w", name="po")
                        if nn == 1:
                            lh = zt_sb[:, kc, tt * 128:(tt + 1) * 128]
                            rh = w_s[:, kc, dc * 512:(dc + 1) * 512]
                        else:
                            lh = zt_sb[:, kc:kc + 2, tt * 128:(tt + 1) * 128]
                            rh = w_s[:, kc:kc + 2, dc * 512:(dc + 1) * 512]
                        nc.tensor.matmul(
                            state[('p', tt, dc)], lhsT=lh, rhs=rh,
                            start=(i == 0), stop=(i == nk - 1), perf_mode=pm)
                    for i in range(nk):
                        atoms.append(lambda i=i, a=a_mm: a(i))
                    atoms.append(lambda tt=tt, dc=dc, oscale=oscale:
                                 nc.vector.tensor_scalar_mul(
                                     state[('o', tt)][:, dc * 512:(dc + 1) * 512],
                                     state[('p', tt, dc)], oscale))
                def a_dma(tt=tt):
                    r0 = q0 + tt * 128
                    nc.sync.dma_start(out=out.ap()[r0:r0 + 128, :],
                                      in_=state[('o', tt)])
                atoms.append(a_dma)
            return atoms

        # ---- attention emission for one chunk, fills interleaved ----
        qt_tiles = [None] * NCH
        zt_tiles = [None] * NCH

        def attention_chunk(ch, fills):
            nkb = 4 * ch + 4
            nkb2 = nkb // 2
            qt_sb = qt_tiles[ch]
            zt_sb = zt_tiles[ch]
            etp = etb_pool if ch == 0 else et8_pool
            etd = BF16 if ch == 0 else FP8
            ett = "etb" if ch == 0 else "et8"
            st = {}

            def emit_S(h, kb2):
                hp, p0 = h // 2, 64 * (h % 2)
                kba, kbb = 2 * kb2, 2 * kb2 + 1
                ja, jb = kba - 4 * ch, kbb - 4 * ch
                ca = 128 * ja if ja > 0 else 0
                cb = 128 * jb if jb > 0 else 0
                oa, ob = (kba % 4) * 128, (kbb % 4) * 128
                s2 = ps_s.tile([128, 2, TCH], F32, tag="s2", name="s2")
                nc.tensor.matmul(
                    s2[:, 0, ca:],
                    lhsT=kt_tiles[kba // 4][p0:p0 + 64, hp, oa:oa + 128],
                    rhs=qt_sb[p0:p0 + 64, hp, ca:],
                    start=True, stop=True, tile_position=(p0, 0))
                nc.tensor.matmul(
                    s2[:, 1, cb:],
                    lhsT=kt_tiles[kbb // 4][p0:p0 + 64, hp, ob:ob + 128],
                    rhs=qt_sb[p0:p0 + 64, hp, cb:],
                    start=True, stop=True, tile_position=(p0, 0))
                et = etp.tile([128, 2, TCH], etd, name="et", tag=ett)
                s2f = s2.rearrange("p a b -> p (a b)")
                etf = et.rearrange("p a b -> p (a b)")
                if ja >= 2:
                    # deep in the diagonal chunk the flat range would span a
                    # large dead zone between the blocks: split the exp
                    nc.scalar.activation(out=etf[:, ca:TCH], in_=s2f[:, ca:TCH],
                                         func=AF.Exp, scale=ESC)
                    nc.scalar.activation(out=etf[:, TCH + cb:],
                                         in_=s2f[:, TCH + cb:],
                                         func=AF.Exp, scale=ESC)
                else:
                    nc.scalar.activation(out=etf[:, ca:], in_=s2f[:, ca:],
                                         func=AF.Exp, scale=ESC)
                if ja >= 0:
                    nc.vector.tensor_mul(et[:, 0, ca:ca + 128],
                                         et[:, 0, ca:ca + 128], tri_sb)
                if jb >= 0:
                    nc.vector.tensor_mul(et[:, 1, cb:cb + 128],
                                         et[:, 1, cb:cb + 128], tri_sb)
                st[(h, kb2)] = et

            def emit_A(h, kb2):
                et = st.pop((h, kb2))
                kba, kbb = 2 * kb2, 2 * kb2 + 1
                ja, jb = kba - 4 * ch, kbb - 4 * ch
                u = st[('u', h)]
                if False and jb < 0:  # AV-DR disabled: chunk 1 needs the
                    # extra 2-MM density to bridge fill round-trip bubbles
                    # at the ch0->ch1 transition, else HAM throttles ~20us
                    # fully off-diagonal pair: one DoubleRow matmul contracts
                    # both key blocks (et slots = the two contraction slabs).
                    # Chunk 3 stays 2-matmul fp8: its windows are ACT-bound,
                    # so the extra PE work is free and keeps the clock warm.
                    nc.tensor.matmul(
                        u, lhsT=v8_tiles[kba // 4][:, (kba % 4) // 2, :, h,
                                                   0:D_HEAD + 1],
                        rhs=et, start=(kba == 0), stop=False, perf_mode=DR)
                    return
                ca = 128 * ja if ja > 0 else 0
                cb = 128 * jb if jb > 0 else 0
                if ch == 0:
                    la = vb0_tile[:, kba % 4, h, :]
                    lb = vb0_tile[:, kbb % 4, h, :]
                else:
                    la = v8_tiles[kba // 4][:, (kba % 4) // 2, 0, h,
                                            0:D_HEAD + 1]
                    lb = v8_tiles[kbb // 4][:, (kbb % 4) // 2, 1, h,
                                            0:D_HEAD + 1]
                nc.tensor.matmul(
                    u[:, ca:], lhsT=la,
                    rhs=et[:, 0, ca:], start=(kba == 0), stop=False)
                nc.tensor.matmul(
                    u[:, cb:], lhsT=lb,
                    rhs=et[:, 1, cb:], start=False, stop=(kbb == nkb - 1))

            def emit_divA(h, split=False):
                # 1/D on ACT (Ln then Exp(-x), both 1-lane [1,512]) straight
                # off the psum row: no DVE involvement at all.
                u = st[('u', h)]
                lnd = sm_pool.tile([1, TCH], F32, name="lnd", tag="lnd",
                                   bufs=3)
                if split:
                    # Epilogue hazard: with an idle ACT queue this Ln starts
                    # the moment its AV stop-matmul's sem fires, and can read
                    # the matmul's last ~128 columns before they finish
                    # draining into PSUM (observed as D=0 -> NaN on cold
                    # first runs). Splitting the read makes the tail part
                    # start one ACT op (~600ns) later with no new sync edges.
                    nc.scalar.activation(out=lnd[:, 0:384],
                                         in_=u[D_HEAD:D_HEAD + 1, 0:384],
                                         func=AF.Ln)
                    nc.scalar.activation(out=lnd[:, 384:],
                                         in_=u[D_HEAD:D_HEAD + 1, 384:],
                                         func=AF.Ln)
                else:
                    nc.scalar.activation(out=lnd, in_=u[D_HEAD:D_HEAD + 1, :],
                                         func=AF.Ln)
                rcp = sm_pool.tile([1, TCH], F32R, name="rcp", tag="rcp",
                                   bufs=3)
                nc.scalar.activation(out=rcp, in_=lnd, func=AF.Exp,
                                     scale=-1.0)
                st[('d', h)] = rcp

            def emit_divB(h):
                hp, p0 = h // 2, 64 * (h % 2)
                u = st.pop(('u', h))
                rcp = st.pop(('d', h))
                db = ps_w.tile([64, TCH], F32, tag="w", name="db")
                nc.tensor.matmul(db, lhsT=ones_sb, rhs=rcp,
                                 start=True, stop=True)
                rb = rb_pool.tile([64, TCH], F32, name="rb", tag="rb")
                nc.vector.tensor_copy(out=rb, in_=db)
                with nc.allow_low_precision("fp8/f32r zt"):
                    nc.vector.tensor_mul(zt_sb[p0:p0 + 64, hp, :],
                                         u[0:D_HEAD, :], rb)

            order = [(h, kb2) for h in range(H) for kb2 in range(nkb2)]
            n = len(order)
            # fills: (front, rate) emitted at fixed rate from window 0; rest
            # paced uniformly over the whole chunk.
            front, frate, rest = fills
            Ff, Fr = len(front), len(rest)
            ffi = fi = 0
            pend_A = []
            pend_ln = []
            pend_div = []

            def emit_pend_A(idx):
                a = pend_A.pop(0)
                emit_A(*a)
                if a[1] == nkb2 - 1:
                    # Ln/Exp deferred one window so they queue BEHIND the
                    # next unit's exp on ACT instead of delaying it
                    pend_ln.append((a[0], idx + 1))

            for idx, (h, kb2) in enumerate(order):
                if kb2 == 0:
                    st[('u', h)] = ps_u.tile([D_HEAD + 1, TCH], F32, name="u",
                                             tag="u")
                emit_S(h, kb2)
                while pend_ln and pend_ln[0][1] <= idx:
                    hd = pend_ln.pop(0)[0]
                    emit_divA(hd)
                    # defer capped so divB (which frees the u slot) is
                    # emitted before A(h+2,0) claims it
                    pend_div.append((hd, idx + min(3, nkb2)))
                wantf = min(Ff, (idx + 1) * frate)
                while ffi < wantf:
                    front[ffi]()
                    ffi += 1
                # pace against n+4 so a few fills remain for the epilogue
                want = (idx + 1) * Fr // (n + 4)
                while fi < want:
                    fills[2][fi]()
                    fi += 1
                # divB of a finished head is deferred so its bcast matmul
                # doesn't make the PE wait on the ACT 1/D chain.
                while pend_div and pend_div[0][1] <= idx:
                    emit_divB(pend_div.pop(0)[0])
                # AV runs two windows behind its scores: exp gets ~2 windows
                # of latency slack, so the PE never waits on ACT.
                if len(pend_A) >= 2:
                    emit_pend_A(idx)
                pend_A.append((h, kb2))
            while pend_A:
                emit_pend_A(n - 1)
            first_flush = True
            while pend_ln:
                hd = pend_ln.pop(0)[0]
                emit_divA(hd, split=first_flush)
                first_flush = False
                pend_div.append((hd, n))
            while ffi < Ff:
                front[ffi]()
                ffi += 1
            while fi < Fr:
                rest[fi]()
                fi += 1
            while pend_div:
                emit_divB(pend_div.pop(0)[0])

        # ---- schedule ----
        # proj(0) upfront. Fills: ch0 <- proj(1); ch1 <- outproj(0)+proj(2);
        # ch2 <- outproj(1)+proj(3).uq; ch3 <- front-loaded proj(3).uk/uv
        # (kt/v of the diagonal chunk, needed from kb2=6) + outproj(2).
        def flat(units):
            return [a for unit in units for a in unit]

        qt_tiles[0] = qt_pool.tile([128, 4, TCH], BF16, name="qt", tag="qt")
        uq0, uk0, uv0 = proj_parts(0, xs_tiles[0], qt_tiles[0])
        for a in flat([uq0[0], uk0[0], uq0[1], uk0[1], uq0[2], uk0[2],
                       uq0[3], uk0[3]] + uv0):
            a()
        ukv3 = None
        for ch in range(NCH):
            if ch == 0:
                zt_tiles[ch] = ztr_pool.tile([128, 4, TCH], F32R, name="ztr",
                                             tag="ztr")
            else:
                zt_tiles[ch] = zt8_pool.tile([128, 4, TCH], FP8, name="zt8",
                                             tag="zt8")
            front, frate, rest = [], 0, []
            if ch >= 1:
                rest += outproj_atoms(ch - 1, zt_tiles[ch - 1])
            if ch + 1 < NCH:
                if ch + 3 < NCH:
                    dma_xs(ch + 3)
                qt_tiles[ch + 1] = qt_pool.tile([128, 4, TCH], BF16,
                                                name="qt", tag="qt")
                uq, uk, uv = proj_parts(ch + 1, xs_tiles[ch + 1],
                                        qt_tiles[ch + 1])
                if ch + 1 < NCH - 1:
                    rest += flat([uq[0], uk[0], uq[1], uk[1], uq[2], uk[2],
                                  uq[3], uk[3]] + uv)
                else:
                    # last chunk: only q-proj ahead of time; kt/v of the
                    # diagonal chunk become chunk-3 fills (uk0+uv paced
                    # early for h0's diagonal blocks, the rest uniform).
                    rest += flat(uq)
                    ukv3 = (flat([uk[0], uv[0], uv[1], uv[2], uv[3]]),
                            flat([uk[1], uk[2], uk[3]]))
            if ch == NCH - 1 and ukv3 is not None:
                front, frate = ukv3[0], 5
                rest = ukv3[1] + rest
            attention_chunk(ch, (front, frate, rest))
        for a in outproj_atoms(NCH - 1, zt_tiles[NCH - 1]):
            a()

    _split_multi_waits(nc)
    return nc


_NC_CACHE = None


def _get_nc():
    global _NC_CACHE
    if _NC_CACHE is None:
        _NC_CACHE = _build()
    return _NC_CACHE


def _make_in_maps(x, W_Q, W_K, W_V, W_O):
    import ml_dtypes
    f8 = ml_dtypes.float8_e4m3fn
    bf = ml_dtypes.bfloat16

    def q8(a):
        return np.clip(np.asarray(a, np.float32), -240, 240).astype(f8)

    x = np.asarray(x, dtype=np.float32)
    xb = x.astype(bf)
    W_Q32 = np.asarray(W_Q, dtype=np.float32) * 32.0
    W_K32 = np.asarray(W_K, dtype=np.float32) * 32.0
    W_V32 = np.asarray(W_V, dtype=np.float32) * 32.0
    W_O = np.asarray(W_O, dtype=np.float32)
    tri = np.triu(np.ones((128, 128), dtype=bf))  # col >= row
    ones1 = np.full((1, 64), 2.0, dtype=np.float32)
    vones_b = np.ones((128, 4, H, 1), dtype=bf)
    vones_8 = np.ones((128, T // 128, H, 1), dtype=f8)

    in_maps = []
    for core in range(8):
        b, g = core // 2, core % 2
        cs = slice(g * HG, (g + 1) * HG)
        xT = np.ascontiguousarray(x[b].T)
        in_maps.append({
            "xT": np.ascontiguousarray(xb[b].T),
            "xT8": q8(xT),
            "wq": np.ascontiguousarray(W_Q32[:, cs]).astype(bf),
            "wk": np.ascontiguousarray(W_K32[:, cs]).astype(bf),
            "wv": np.ascontiguousarray(W_V32[:, cs]).astype(bf),
            "wq8": q8(W_Q32[:, cs]),
            "wk8": q8(W_K32[:, cs]),
            "wv8": q8(W_V32[:, cs]),
            "wo": np.ascontiguousarray(W_O[cs, :]),
            "wo8": q8(W_O[cs, :] * 4.0),
            "tri": tri, "ones1": ones1,
            "vones_b": vones_b, "vones_8": vones_8,
        })
    return in_maps


def kernel(x, W_Q, W_K, W_V, W_O):
    in_maps = _make_in_maps(x, W_Q, W_K, W_V, W_O)
    nc = _get_nc()
    res = run_bass_kernel_spmd(nc, in_maps, core_ids=list(range(8)))
    outs = [res.results[c]["out"] for c in range(8)]
    full = np.stack([outs[2 * b] + outs[2 * b + 1] for b in range(B)], axis=0)
    return full


# revision 39
# speedup vs baseline: 1.0107x; 1.0107x over previous
"""Causal multi-head attention on 8 TRN2 NeuronCores.

Problem: B=4, T=2048, d_model=1024, 16 heads x 64. out = softmax(causal(QK^T)/8) V Wo.

Sharding (tensor-parallel heads x data-parallel batch):
  core c -> batch b = c//2, head group g = c%2 (8 heads each).
  Each core computes a partial output  z_g[b] @ Wo[g] : [2048, 1024];
  host sums the two head-group partials per batch.

Per-core kernel. PE and ACT are co-critical (~210us each): the ACT exp
stream (one exp per score element + the Ln->Exp 1/D chain) is a hard floor,
so PE work is trimmed to just above it -- going lower only re-throttles the
PE clock (HAM) during the idle gaps and makes everything slower:
  - Projections stay bf16 (they pace the pipeline and keep the PE warm).
  - AV off-diagonal key-block pairs run as single fp8e4 DoubleRow matmuls
    (HW-measured 2.0x: both key blocks contract in one 512-column stream).
    The ACT exp writes those attention weights straight to fp8; V tiles are
    kept as fp8 32*V pairs (slot stride 576B satisfies DR's 16B rule).
    Diagonal pairs stay 2-matmul (fp8 non-DR ch1-3, bf16 ch0) so causal
    dead zones are never streamed; chunk 0 stays bf16 because its small-Neff
    rows are sensitive to weight/V noise. Mixed DR/non-DR psum accumulation
    groups are HW-validated.
  - Output projection chunks 1-3: zt as fp8 64z x Wo as fp8 4*Wo, DoubleRow
    over head-dim pairs; psum = 256*out, rescaled in the DVE copy. Chunk 0
    stays f32r (zt = 64z there too; copy rescales by 1/64).
  Scale plumbing: V copies apply x32 (e4m3's normal range starts at 2^-6);
  the 1/D broadcast uses ones=2.0 so zt = u * (2/D) = 64z uniformly.
Baseline schedule retained: per-head attention pipeline, AV two windows
behind scores, proj/outproj fills interleaved, deferred 1/D chain.
"""
import numpy as np

import concourse.bass as bass
import concourse.tile as tile
import concourse.mybir as mybir
from concourse.vector_clock import ScopedClock
from concourse.bass_utils import run_bass_kernel_spmd

D_MODEL = 1024
D_HEAD = 64
B = 4
T = 2048
H = 8              # heads per core
HG = H * D_HEAD    # 512 head-dim columns per core
TCH = 512          # q/t chunk
NCH = T // TCH     # 4
NDM = D_MODEL // 128  # 8 d_model chunks

F32R = mybir.dt.float32r
F32 = mybir.dt.float32
BF16 = mybir.dt.bfloat16
FP8 = mybir.dt.float8e4
AF = mybir.ActivationFunctionType
DR = mybir.MatmulPerfMode.DoubleRow
ESC = 0.125 / 1024.0   # exp scale: scores are (32q)(32k) = 1024*S


class _TC(tile.TileContext):
    """TileContext whose tail drain carries no sem waits (this walrus build
    rejects >1 sync wait per instruction and any wait on a Drain)."""

    def _drain_and_barrier(self, tick_clock, wait_clock):
        drain_inst = self.nc.sync.drain()
        wait_clock.add_sem_waits(
            drain_inst.ins, ScopedClock({None: tick_clock.global_clock})
        )
        si = drain_inst.ins.sync_info
        waits = list(si.on_wait) if si is not None else []
        if waits:
            drain_inst.ins.sync_info = mybir.SyncInfo(
                on_wait=[], on_update=list(si.on_update)
            )
            for w in waits:
                nop = self.nc.sync.nop(nofuse=True)
                nop.ins.sync_info = mybir.SyncInfo(on_wait=[w], on_update=[])
        self.nc.all_engine_barrier()
        popped = self.nc._tile_sem_poison_stack.pop()
        assert popped is self._sem_poison
        self.nc.clear_and_free_semaphores(list(self.sems.allocated().values()))
        self.nc.all_engine_barrier()


def _split_multi_waits(nc):
    """Move all-but-one sem wait of every instruction onto same-engine NOPs."""
    cnt = 0
    for f in nc.m.functions:
        for b in f.blocks:
            new = []
            for inst in b.instructions:
                si = inst.sync_info
                if si is not None and si.on_wait is not None:
                    waits = list(si.on_wait)
                    max_keep = 0 if inst.opcode == "Drain" else 1
                    if len(waits) > max_keep:
                        keep = waits[len(waits) - max_keep:] if max_keep else []
                        spill = waits[: len(waits) - max_keep]
                        for w in spill:
                            nop = mybir.InstNoOp(
                                name=f"I-wsplit-{cnt}", engine=inst.engine,
                                ins=[], outs=[],
                            )
                            nop.sync_info = mybir.SyncInfo(
                                on_wait=[w], on_update=[]
                            )
                            new.append(nop)
                            cnt += 1
                        inst.sync_info = mybir.SyncInfo(
                            on_wait=keep, on_update=list(si.on_update)
                        )
                new.append(inst)
            b.instructions = new
    return cnt


def _build():
    nc = bass.Bass("TRN2", target_bir_lowering=False)
    xT = nc.dram_tensor("xT", (D_MODEL, T), BF16, kind="ExternalInput")
    xT8 = nc.dram_tensor("xT8", (D_MODEL, T), FP8, kind="ExternalInput")
    wq = nc.dram_tensor("wq", (D_MODEL, HG), BF16, kind="ExternalInput")
    wk = nc.dram_tensor("wk", (D_MODEL, HG), BF16, kind="ExternalInput")
    wv = nc.dram_tensor("wv", (D_MODEL, HG), BF16, kind="ExternalInput")
    wq8 = nc.dram_tensor("wq8", (D_MODEL, HG), FP8, kind="ExternalInput")
    wk8 = nc.dram_tensor("wk8", (D_MODEL, HG), FP8, kind="ExternalInput")
    wv8 = nc.dram_tensor("wv8", (D_MODEL, HG), FP8, kind="ExternalInput")
    wo = nc.dram_tensor("wo", (HG, D_MODEL), F32R, kind="ExternalInput")
    wo8 = nc.dram_tensor("wo8", (HG, D_MODEL), FP8, kind="ExternalInput")
    tri = nc.dram_tensor("tri", (128, 128), BF16, kind="ExternalInput")
    ones1 = nc.dram_tensor("ones1", (1, 64), F32R, kind="ExternalInput")
    vones_b = nc.dram_tensor("vones_b", (128, 4, H, 1), BF16,
                             kind="ExternalInput")
    vones_8 = nc.dram_tensor("vones_8", (128, T // 128, H, 1), FP8,
                             kind="ExternalInput")
    out = nc.dram_tensor("out", (T, D_MODEL), F32, kind="ExternalOutput")

    from contextlib import ExitStack
    with _TC(nc) as tc, ExitStack() as ctx:
        consts = ctx.enter_context(tc.tile_pool(name="consts", bufs=1))
        xs_pool = ctx.enter_context(tc.tile_pool(name="xs", bufs=3))
        kt_pool = ctx.enter_context(tc.tile_pool(name="kt", bufs=1))
        v_pool = ctx.enter_context(tc.tile_pool(name="v", bufs=1))
        qt_pool = ctx.enter_context(tc.tile_pool(name="qt", bufs=3))
        ztr_pool = ctx.enter_context(tc.tile_pool(name="ztr", bufs=1))
        zt8_pool = ctx.enter_context(tc.tile_pool(name="zt8", bufs=3))
        etb_pool = ctx.enter_context(tc.tile_pool(name="etb", bufs=4))
        et8_pool = ctx.enter_context(tc.tile_pool(name="et8", bufs=6))
        sm_pool = ctx.enter_context(tc.tile_pool(name="sm", bufs=2))
        rb_pool = ctx.enter_context(tc.tile_pool(name="rb", bufs=2))
        ou_pool = ctx.enter_context(tc.tile_pool(name="ou", bufs=2))
        # PSUM: 8 banks. s2 tiles [128,2,512]f32 = 2 banks x 2 bufs;
        # u tiles 1 bank x 2; w tiles (proj/outproj/bcast) 1 x 2.
        ps_s = ctx.enter_context(tc.tile_pool(name="ps_s", bufs=2, space="PSUM"))
        ps_u = ctx.enter_context(tc.tile_pool(name="ps_u", bufs=2, space="PSUM"))
        ps_w = ctx.enter_context(tc.tile_pool(name="ps_w", bufs=2, space="PSUM"))

        xT_r = xT.ap().rearrange("(c p) t -> p c t", p=128)
        xT8_r = xT8.ap().rearrange("(c p) t -> p c t", p=128)

        xs_tiles = [None] * NCH

        def dma_xs(ch):
            if ch == 1:
                xs_tiles[ch] = xs_pool.tile([128, NDM, TCH], FP8, name="xs8",
                                            tag="xs8")
                for c in range(0, NDM, 4):
                    nc.sync.dma_start(out=xs_tiles[ch][:, c:c + 4, :],
                                      in_=xT8_r[:, c:c + 4,
                                               ch * TCH:(ch + 1) * TCH])
            else:
                xs_tiles[ch] = xs_pool.tile([128, NDM, TCH], BF16, name="xs",
                                            tag="xs")
                for c in range(0, NDM, 2):
                    nc.sync.dma_start(out=xs_tiles[ch][:, c:c + 2, :],
                                      in_=xT_r[:, c:c + 2,
                                               ch * TCH:(ch + 1) * TCH])

        # resident weights / constants, DMA'd in need-order on the serial
        # sync queue: proj(0)'s bf16 wq/x/wk first, then chunk-0's fill
        # inputs (fp8 x1 + fp8 q/k weights), then the V weights, constants,
        # x2; wo/wo8 are deferred to chunk 1 (first read mid-chunk 2).
        wq_sb = consts.tile([128, NDM, HG], BF16)
        xs0 = xs_pool.tile([128, NDM, TCH], BF16, name="xs", tag="xs")
        wk_sb = consts.tile([128, NDM, HG], BF16)
        wv_sb = consts.tile([128, NDM, HG], BF16)
        wq_r = wq.ap().rearrange("(c p) n -> p c n", p=128)
        wk_r = wk.ap().rearrange("(c p) n -> p c n", p=128)
        # 2-chunk DMAs: the sync queue costs ~600ns per descriptor no matter
        # the size, so fewer+bigger descriptors feed proj(0) faster
        for c in range(0, NDM, 2):
            nc.sync.dma_start(out=wq_sb[:, c:c + 2, :], in_=wq_r[:, c:c + 2, :])
            nc.sync.dma_start(out=xs0[:, c:c + 2, :],
                              in_=xT_r[:, c:c + 2, 0:TCH])
            nc.sync.dma_start(out=wk_sb[:, c:c + 2, :], in_=wk_r[:, c:c + 2, :])
        xs_tiles[0] = xs0
        wq8_sb = consts.tile([128, NDM, HG], FP8)
        wk8_sb = consts.tile([128, NDM, HG], FP8)
        wv8_sb = consts.tile([128, NDM, HG], FP8)
        dma_xs(1)
        nc.sync.dma_start(out=wq8_sb,
                          in_=wq8.ap().rearrange("(c p) n -> p c n", p=128))
        nc.sync.dma_start(out=wk8_sb,
                          in_=wk8.ap().rearrange("(c p) n -> p c n", p=128))
        nc.sync.dma_start(out=wv_sb,
                          in_=wv.ap().rearrange("(c p) n -> p c n", p=128))
        nc.sync.dma_start(out=wv8_sb,
                          in_=wv8.ap().rearrange("(c p) n -> p c n", p=128))
        tri_sb = consts.tile([128, 128], BF16)
        nc.sync.dma_start(out=tri_sb, in_=tri.ap())
        ones_sb = consts.tile([1, 64], F32R)
        nc.sync.dma_start(out=ones_sb, in_=ones1.ap())
        wo_sb = consts.tile([128, HG // 128, D_MODEL], F32R)
        wo8_sb = consts.tile([128, HG // 128, D_MODEL], FP8)

        # per-chunk K^T tiles [pair-packed 128, head-pair, t-in-chunk];
        # fp8 V pair-tiles [128, pair, slot, H, 72] (slot stride 576 % 16 == 0
        # for DoubleRow; col 64 = ones so row 64 of U accumulates the softmax
        # denominator); bf16 V tile for chunk 0 only.
        kt_tiles = [kt_pool.tile([128, 4, TCH], BF16, name=f"kt{i}", tag=f"kt{i}")
                    for i in range(NCH)]
        v8_tiles = [v_pool.tile([128, 2, 2, H, 72], FP8, name=f"v8_{i}",
                                tag=f"v8_{i}") for i in range(NCH)]
        vb0_tile = consts.tile([128, 4, H, D_HEAD + 1], BF16)
        vo8_r = vones_8.ap().rearrange("p (a c d) h o -> p a c d h o", c=2, d=2)
        for i in range(NCH):
            nc.sync.dma_start(out=v8_tiles[i][:, :, :, :, D_HEAD:D_HEAD + 1],
                              in_=vo8_r[:, i])
        nc.sync.dma_start(out=vb0_tile[:, :, :, D_HEAD:], in_=vones_b.ap())
        dma_xs(2)
        nc.sync.dma_start(out=wo_sb,
                          in_=wo.ap().rearrange("(c p) n -> p c n", p=128))
        nc.sync.dma_start(out=wo8_sb,
                          in_=wo8.ap().rearrange("(c p) n -> p c n", p=128))

        # ---- fill atoms: proj / outproj work split into single-PE-op units
        # Weights are pre-scaled 32x on the host (both dtypes) so all psum
        # projections are 32*(q|k|v) regardless of chunk dtype.
        def proj_parts(ch, xs, qt_sb):
            """Q^T,K^T per dqc and V per tt for chunk ch; returns per-unit
            atom lists. Each atom = 1 PE mm or 1 DVE copy. Chunks 1-2 run
            fp8 DoubleRow (they fill the PE-bound chunks 0-1); chunks 0/3
            stay bf16 (0 for small-Neff accuracy, 3 for fill mass in the
            ACT-bound late chunks)."""
            state = {}
            uq, uk, uv = [], [], []
            dr = ch == 1
            if dr:
                wq_s, wk_s, wv_s = wq8_sb, wk8_sb, wv8_sb
                steps = [(2 * i, 2) for i in range(NDM // 2)]
            else:
                wq_s, wk_s, wv_s = wq_sb, wk_sb, wv_sb
                steps = [(c, 1) for c in range(NDM)]
            ns = len(steps)
            pm = DR if dr else None

            def w_lhs(w, i, dqc):
                c, nn = steps[i]
                if nn == 1:
                    return w[:, c, dqc * 128:(dqc + 1) * 128]
                return w[:, c:c + 2, dqc * 128:(dqc + 1) * 128]

            def x_rhs(i):
                c, nn = steps[i]
                return xs[:, c, :] if nn == 1 else xs[:, c:c + 2, :]

            for dqc in range(4):
                unit = []
                def a_q(i, dqc=dqc):
                    if i == 0:
                        state[('q', dqc)] = ps_w.tile([128, TCH], F32, tag="w",
                                                      name="pq")
                    nc.tensor.matmul(
                        state[('q', dqc)], lhsT=w_lhs(wq_s, i, dqc),
                        rhs=x_rhs(i), start=(i == 0), stop=(i == ns - 1),
                        perf_mode=pm)
                for i in range(ns):
                    unit.append(lambda i=i, a=a_q: a(i))
                unit.append(lambda dqc=dqc: nc.vector.tensor_copy(
                    out=qt_sb[:, dqc, :], in_=state[('q', dqc)]))
                uq.append(unit)
                unit = []
                def a_k(i, dqc=dqc):
                    if i == 0:
                        state[('k', dqc)] = ps_w.tile([128, TCH], F32, tag="w",
                                                      name="pk")
                    nc.tensor.matmul(
                        state[('k', dqc)], lhsT=w_lhs(wk_s, i, dqc),
                        rhs=x_rhs(i), start=(i == 0), stop=(i == ns - 1),
                        perf_mode=pm)
                for i in range(ns):
                    unit.append(lambda i=i, a=a_k: a(i))
                unit.append(lambda dqc=dqc: nc.vector.tensor_copy(
                    out=kt_tiles[ch][:, dqc, :], in_=state[('k', dqc)]))
                uk.append(unit)
            for tt in range(4):
                unit = []
                def a_v(i, tt=tt):
                    c, nn = steps[i]
                    if i == 0:
                        state[('v', tt)] = ps_w.tile([128, HG], F32, tag="w",
                                                     name="pv")
                    if nn == 1:
                        nc.tensor.matmul(
                            state[('v', tt)],
                            lhsT=xs[:, c, tt * 128:(tt + 1) * 128],
                            rhs=wv_s[:, c, :], start=(i == 0),
                            stop=(i == ns - 1))
                    else:
                        nc.tensor.matmul(
                            state[('v', tt)],
                            lhsT=xs[:, c:c + 2, tt * 128:(tt + 1) * 128],
                            rhs=wv_s[:, c:c + 2, :], start=(i == 0),
                            stop=(i == ns - 1), perf_mode=DR)
                for i in range(ns):
                    unit.append(lambda i=i, a=a_v: a(i))
                # psum already holds 32*V; fp8 copy always, bf16 copy
                # additionally for chunk 0 (its diagonal AV)
                def c_v8(tt=tt, ch=ch):
                    with nc.allow_low_precision("fp8 32V tiles"):
                        nc.vector.tensor_copy(
                            out=v8_tiles[ch][:, tt // 2, tt % 2, :, 0:D_HEAD],
                            in_=state[('v', tt)].rearrange("p (h d) -> p h d",
                                                           h=H))
                unit.append(c_v8)
                if ch == 0:
                    unit.append(lambda tt=tt: nc.vector.tensor_copy(
                        out=vb0_tile[:, tt, :, 0:D_HEAD],
                        in_=state[('v', tt)].rearrange("p (h d) -> p h d",
                                                       h=H)))
                uv.append(unit)
            return uq, uk, uv

        def outproj_atoms(ch, zt_sb):
            atoms = []
            q0 = ch * TCH
            state = {}
            if ch == 0:
                ksteps = [(kc, 1) for kc in range(4)]
                w_s, pm, oscale = wo_sb, None, 1.0 / 64
            elif ch == NCH - 1:
                # epilogue outproj: fastest (DoubleRow)
                ksteps = [(0, 2), (2, 2)]
                w_s, pm, oscale = wo8_sb, DR, 1.0 / 256
            else:
                # fp8 non-DR: same psum scale as DR but twice the matmuls --
                # these fill the ACT-bound chunks 2-3 and keep the PE warm
                ksteps = [(kc, 1) for kc in range(4)]
                w_s, pm, oscale = wo8_sb, None, 1.0 / 256
            nk = len(ksteps)
            for tt in range(4):
                def a_alloc(tt=tt):
                    state[('o', tt)] = ou_pool.tile([128, D_MODEL], F32,
                                                    name="o_sb", tag="o")
                atoms.append(a_alloc)
                for dc in range(2):
                    def a_mm(i, tt=tt, dc=dc):
                        kc, nn = ksteps[i]
                        if i == 0:
                            state[('p', tt, dc)] = ps_w.tile(
                                [128, 512], F32, tag="w", name="po")
                        if nn == 1:
                            lh = zt_sb[:, kc, tt * 128:(tt + 1) * 128]
                            rh = w_s[:, kc, dc * 512:(dc + 1) * 512]
                        else:
                            lh = zt_sb[:, kc:kc + 2, tt * 128:(tt + 1) * 128]
                            rh = w_s[:, kc:kc + 2, dc * 512:(dc + 1) * 512]
                        nc.tensor.matmul(
                            state[('p', tt, dc)], lhsT=lh, rhs=rh,
                            start=(i == 0), stop=(i == nk - 1), perf_mode=pm)
                    for i in range(nk):
                        atoms.append(lambda i=i, a=a_mm: a(i))
                    atoms.append(lambda tt=tt, dc=dc, oscale=oscale:
                                 nc.vector.tensor_scalar_mul(
                                     state[('o', tt)][:, dc * 512:(dc + 1) * 512],
                                     state[('p', tt, dc)], oscale))
                def a_dma(tt=tt):
                    r0 = q0 + tt * 128
                    nc.sync.dma_start(out=out.ap()[r0:r0 + 128, :],
                                      in_=state[('o', tt)])
                atoms.append(a_dma)
            return atoms

        # ---- attention emission for one chunk, fills interleaved ----
        qt_tiles = [None] * NCH
        zt_tiles = [None] * NCH

        def attention_chunk(ch, fills):
            nkb = 4 * ch + 4
            nkb2 = nkb // 2
            qt_sb = qt_tiles[ch]
            zt_sb = zt_tiles[ch]
            etp = etb_pool if ch == 0 else et8_pool
            etd = BF16 if ch == 0 else FP8
            ett = "etb" if ch == 0 else "et8"
            st = {}

            def emit_S(h, kb2):
                hp, p0 = h // 2, 64 * (h % 2)
                kba, kbb = 2 * kb2, 2 * kb2 + 1
                ja, jb = kba - 4 * ch, kbb - 4 * ch
                ca = 128 * ja if ja > 0 else 0
                cb = 128 * jb if jb > 0 else 0
                oa, ob = (kba % 4) * 128, (kbb % 4) * 128
                s2 = ps_s.tile([128, 2, TCH], F32, tag="s2", name="s2")
                nc.tensor.matmul(
                    s2[:, 0, ca:],
                    lhsT=kt_tiles[kba // 4][p0:p0 + 64, hp, oa:oa + 128],
                    rhs=qt_sb[p0:p0 + 64, hp, ca:],
                    start=True, stop=True, tile_position=(p0, 0))
                nc.tensor.matmul(
                    s2[:, 1, cb:],
                    lhsT=kt_tiles[kbb // 4][p0:p0 + 64, hp, ob:ob + 128],
                    rhs=qt_sb[p0:p0 + 64, hp, cb:],
                    start=True, stop=True, tile_position=(p0, 0))
                et = etp.tile([128, 2, TCH], etd, name="et", tag=ett)
                s2f = s2.rearrange("p a b -> p (a b)")
                etf = et.rearrange("p a b -> p (a b)")
                if ja >= 2:
                    # deep in the diagonal chunk the flat range would span a
                    # large dead zone between the blocks: split the exp
                    nc.scalar.activation(out=etf[:, ca:TCH], in_=s2f[:, ca:TCH],
                                         func=AF.Exp, scale=ESC)
                    nc.scalar.activation(out=etf[:, TCH + cb:],
                                         in_=s2f[:, TCH + cb:],
                                         func=AF.Exp, scale=ESC)
                else:
                    nc.scalar.activation(out=etf[:, ca:], in_=s2f[:, ca:],
                                         func=AF.Exp, scale=ESC)
                if ja >= 0:
                    nc.vector.tensor_mul(et[:, 0, ca:ca + 128],
                                         et[:, 0, ca:ca + 128], tri_sb)
                if jb >= 0:
                    nc.vector.tensor_mul(et[:, 1, cb:cb + 128],
                                         et[:, 1, cb:cb + 128], tri_sb)
                st[(h, kb2)] = et

            def emit_A(h, kb2):
                et = st.pop((h, kb2))
                kba, kbb = 2 * kb2, 2 * kb2 + 1
                ja, jb = kba - 4 * ch, kbb - 4 * ch
                u = st[('u', h)]
                if ch == 1 and jb < 0:
                    # fully off-diagonal pair: one DoubleRow matmul contracts
                    # both key blocks (et slots = the two contraction slabs).
                    # Chunk 3 stays 2-matmul fp8: its windows are ACT-bound,
                    # so the extra PE work is free and keeps the clock warm.
                    nc.tensor.matmul(
                        u, lhsT=v8_tiles[kba // 4][:, (kba % 4) // 2, :, h,
                                                   0:D_HEAD + 1],
                        rhs=et, start=(kba == 0), stop=False, perf_mode=DR)
                    return
                ca = 128 * ja if ja > 0 else 0
                cb = 128 * jb if jb > 0 else 0
                if ch == 0:
                    la = vb0_tile[:, kba % 4, h, :]
                    lb = vb0_tile[:, kbb % 4, h, :]
                else:
                    la = v8_tiles[kba // 4][:, (kba % 4) // 2, 0, h,
                                            0:D_HEAD + 1]
                    lb = v8_tiles[kbb // 4][:, (kbb % 4) // 2, 1, h,
                                            0:D_HEAD + 1]
                nc.tensor.matmul(
                    u[:, ca:], lhsT=la,
                    rhs=et[:, 0, ca:], start=(kba == 0), stop=False)
                nc.tensor.matmul(
                    u[:, cb:], lhsT=lb,
                    rhs=et[:, 1, cb:], start=False, stop=(kbb == nkb - 1))

            def emit_divA(h, split=False):
                # 1/D on ACT (Ln then Exp(-x), both 1-lane [1,512]) straight
                # off the psum row: no DVE involvement at all.
                u = st[('u', h)]
                lnd = sm_pool.tile([1, TCH], F32, name="lnd", tag="lnd",
                                   bufs=3)
                if split:
                    # Epilogue hazard: with an idle ACT queue this Ln starts
                    # the moment its AV stop-matmul's sem fires, and can read
                    # the matmul's last ~128 columns before they finish
                    # draining into PSUM (observed as D=0 -> NaN on cold
                    # first runs). Splitting the read makes the tail part
                    # start one ACT op (~600ns) later with no new sync edges.
                    nc.scalar.activation(out=lnd[:, 0:384],
                                         in_=u[D_HEAD:D_HEAD + 1, 0:384],
                                         func=AF.Ln)
                    nc.scalar.activation(out=lnd[:, 384:],
                                         in_=u[D_HEAD:D_HEAD + 1, 384:],
                                         func=AF.Ln)
                else:
                    nc.scalar.activation(out=lnd, in_=u[D_HEAD:D_HEAD + 1, :],
                                         func=AF.Ln)
                rcp = sm_pool.tile([1, TCH], F32R, name="rcp", tag="rcp",
                                   bufs=3)
                nc.scalar.activation(out=rcp, in_=lnd, func=AF.Exp,
                                     scale=-1.0)
                st[('d', h)] = rcp

            def emit_divB(h):
                hp, p0 = h // 2, 64 * (h % 2)
                u = st.pop(('u', h))
                rcp = st.pop(('d', h))
                db = ps_w.tile([64, TCH], F32, tag="w", name="db")
                nc.tensor.matmul(db, lhsT=ones_sb, rhs=rcp,
                                 start=True, stop=True)
                rb = rb_pool.tile([64, TCH], F32, name="rb", tag="rb")
                nc.vector.tensor_copy(out=rb, in_=db)
                with nc.allow_low_precision("fp8/f32r zt"):
                    nc.vector.tensor_mul(zt_sb[p0:p0 + 64, hp, :],
                                         u[0:D_HEAD, :], rb)

            order = [(h, kb2) for h in range(H) for kb2 in range(nkb2)]
            n = len(order)
            # fills: (front, rate) emitted at fixed rate from window 0; rest
            # paced uniformly over the whole chunk.
            front, frate, rest = fills
            Ff, Fr = len(front), len(rest)
            ffi = fi = 0
            pend_A = []
            pend_ln = []
            pend_div = []

            def emit_pend_A(idx):
                a = pend_A.pop(0)
                emit_A(*a)
                if a[1] == nkb2 - 1:
                    # Ln/Exp deferred one window so they queue BEHIND the
                    # next unit's exp on ACT instead of delaying it
                    pend_ln.append((a[0], idx + 1))

            for idx, (h, kb2) in enumerate(order):
                if kb2 == 0:
                    st[('u', h)] = ps_u.tile([D_HEAD + 1, TCH], F32, name="u",
                                             tag="u")
                emit_S(h, kb2)
                while pend_ln and pend_ln[0][1] <= idx:
                    hd = pend_ln.pop(0)[0]
                    emit_divA(hd)
                    # defer capped so divB (which frees the u slot) is
                    # emitted before A(h+2,0) claims it
                    pend_div.append((hd, idx + min(3, nkb2)))
                wantf = min(Ff, (idx + 1) * frate)
                while ffi < wantf:
                    front[ffi]()
                    ffi += 1
                # pace against n+4 so a few fills remain for the epilogue
                want = (idx + 1) * Fr // (n + 4)
                while fi < want:
                    fills[2][fi]()
                    fi += 1
                # divB of a finished head is deferred so its bcast matmul
                # doesn't make the PE wait on the ACT 1/D chain.
                while pend_div and pend_div[0][1] <= idx:
                    emit_divB(pend_div.pop(0)[0])
                # AV runs two windows behind its scores: exp gets ~2 windows
                # of latency slack, so the PE never waits on ACT.
                if len(pend_A) >= 2:
                    emit_pend_A(idx)
                pend_A.append((h, kb2))
            while pend_A:
                emit_pend_A(n - 1)
            first_flush = True
            while pend_ln:
                hd = pend_ln.pop(0)[0]
                emit_divA(hd, split=first_flush)
                first_flush = False
                pend_div.append((hd, n))
            while ffi < Ff:
                front[ffi]()
                ffi += 1
            while fi < Fr:
                rest[fi]()
                fi += 1
            while pend_div:
                emit_divB(pend_div.pop(0)[0])

        # ---- schedule ----
        # proj(0) upfront. Fills: ch0 <- proj(1); ch1 <- outproj(0)+proj(2);
        # ch2 <- outproj(1)+proj(3).uq; ch3 <- front-loaded proj(3).uk/uv
        # (kt/v of the diagonal chunk, needed from kb2=6) + outproj(2).
        def flat(units):
            return [a for unit in units for a in unit]

        qt_tiles[0] = qt_pool.tile([128, 4, TCH], BF16, name="qt", tag="qt")
        uq0, uk0, uv0 = proj_parts(0, xs_tiles[0], qt_tiles[0])
        for a in flat([uq0[0], uk0[0], uq0[1], uk0[1], uq0[2], uk0[2],
                       uq0[3], uk0[3]] + uv0):
            a()
        ukv3 = None
        for ch in range(NCH):
            if ch == 0:
                zt_tiles[ch] = ztr_pool.tile([128, 4, TCH], F32R, name="ztr",
                                             tag="ztr")
            else:
                zt_tiles[ch] = zt8_pool.tile([128, 4, TCH], FP8, name="zt8",
                                             tag="zt8")
            front, frate, rest = [], 0, []
            if ch >= 1:
                rest += outproj_atoms(ch - 1, zt_tiles[ch - 1])
            if ch + 1 < NCH:
                if ch + 3 < NCH:
                    dma_xs(ch + 3)
                qt_tiles[ch + 1] = qt_pool.tile([128, 4, TCH], BF16,
                                                name="qt", tag="qt")
                uq, uk, uv = proj_parts(ch + 1, xs_tiles[ch + 1],
                                        qt_tiles[ch + 1])
                if ch + 1 < NCH - 1:
                    rest += flat([uq[0], uk[0], uq[1], uk[1], uq[2], uk[2],
                                  uq[3], uk[3]] + uv)
                else:
                    # last chunk: only q-proj ahead of time; kt/v of the
                    # diagonal chunk become chunk-3 fills (uk0+uv paced
                    # early for h0's diagonal blocks, the rest uniform).
                    rest += flat(uq)
                    ukv3 = (flat([uk[0], uv[0], uv[1], uv[2], uv[3]]),
                            flat([uk[1], uk[2], uk[3]]))
            if ch == NCH - 1 and ukv3 is not None:
                front, frate = ukv3[0], 5
                rest = ukv3[1] + rest
            attention_chunk(ch, (front, frate, rest))
        for a in outproj_atoms(NCH - 1, zt_tiles[NCH - 1]):
            a()

    _split_multi_waits(nc)
    return nc


_NC_CACHE = None


def _get_nc():
    global _NC_CACHE
    if _NC_CACHE is None:
        _NC_CACHE = _build()
    return _NC_CACHE


def _make_in_maps(x, W_Q, W_K, W_V, W_O):
    import ml_dtypes
    f8 = ml_dtypes.float8_e4m3fn
    bf = ml_dtypes.bfloat16

    def q8(a):
        return np.clip(np.asarray(a, np.float32), -240, 240).astype(f8)

    x = np.asarray(x, dtype=np.float32)
    xb = x.astype(bf)
    W_Q32 = np.asarray(W_Q, dtype=np.float32) * 32.0
    W_K32 = np.asarray(W_K, dtype=np.float32) * 32.0
    W_V32 = np.asarray(W_V, dtype=np.float32) * 32.0
    W_O = np.asarray(W_O, dtype=np.float32)
    tri = np.triu(np.ones((128, 128), dtype=bf))  # col >= row
    ones1 = np.full((1, 64), 2.0, dtype=np.float32)
    vones_b = np.ones((128, 4, H, 1), dtype=bf)
    vones_8 = np.ones((128, T // 128, H, 1), dtype=f8)

    in_maps = []
    for core in range(8):
        b, g = core // 2, core % 2
        cs = slice(g * HG, (g + 1) * HG)
        xT = np.ascontiguousarray(x[b].T)
        in_maps.append({
            "xT": np.ascontiguousarray(xb[b].T),
            "xT8": q8(xT),
            "wq": np.ascontiguousarray(W_Q32[:, cs]).astype(bf),
            "wk": np.ascontiguousarray(W_K32[:, cs]).astype(bf),
            "wv": np.ascontiguousarray(W_V32[:, cs]).astype(bf),
            "wq8": q8(W_Q32[:, cs]),
            "wk8": q8(W_K32[:, cs]),
            "wv8": q8(W_V32[:, cs]),
            "wo": np.ascontiguousarray(W_O[cs, :]),
            "wo8": q8(W_O[cs, :] * 4.0),
            "tri": tri, "ones1": ones1,
            "vones_b": vones_b, "vones_8": vones_8,
        })
    return in_maps


def kernel(x, W_Q, W_K, W_V, W_O):
    in_maps = _make_in_maps(x, W_Q, W_K, W_V, W_O)
    nc = _get_nc()
    res = run_bass_kernel_spmd(nc, in_maps, core_ids=list(range(8)))
    outs = [res.results[c]["out"] for c in range(8)]
    full = np.stack([outs[2 * b] + outs[2 * b + 1] for b in range(B)], axis=0)
    return full
